# revision 32
# baseline (speedup 1.0000x reference)
"""BigBird attention (B=2, T=8193, D=1024, H=8, DK=DV=64, BS=128) on 8
Trainium2 NeuronCores.

Sharding: core c handles batch c//4, sequence quarter c%4 (2048 tokens).
Each core processes its quarter in two 1024-token halves. Block-local
attention runs on-device with a 1-block halo (zero-padded at the sequence
edges, faithful to the reference's zero-block padding). The single global
token's row (query 0 attending everything) is reduced on the host from
k/v tensors exported by each core; the global COLUMN (every block attending
token 0) is handled on-device by treating token 0 as an extra k-tile whose
"ones" column is masked to its first row.

Precision plan: everything runs in bf16 with fp32 PSUM accumulation. The
host ships x pre-transposed and pre-cast to bf16 (half the DMA bytes, no
PE transposes, no staging casts); weights ship as bf16 too. On TRN2 a
bf16 stationary load is half the passes of f32r and bf16 matmuls avoid
f32r's 4-cycles-per-row penalty below 256 columns.
"""

import os
import numpy as np

H, DK, DV, BS = 8, 64, 64, 128
B, T, D = 2, 8193, 1024
INNER = H * DK            # 512
QUART = 2048              # tokens per core
NHALF = 1024              # tokens per half
NT = 11                   # slab tiles per half: [x0pad | haloL | 8 blocks | haloR]
SLAB = NT * 128           # 1408
VW = 66                   # v column group width (64 values + 2 ones cols)
SCALE = 1.0 / 8.0         # 1/sqrt(DK)

_CACHE = {}


def _build_nc():
    import concourse.bacc as bacc
    import concourse.mybir as mybir
    import concourse.tile as tile
    from concourse.masks import make_identity

    F32 = mybir.dt.float32
    BF16 = mybir.dt.bfloat16
    FP8 = mybir.dt.float8e4
    EXPF = mybir.ActivationFunctionType.Exp
    MUL = mybir.AluOpType.mult
    ADD = mybir.AluOpType.add

    nc = bacc.Bacc("TRN2", target_bir_lowering=False, debug=False, num_devices=8)

    # x transposed on host: [D, 2432] = [D, 2304 slab tokens | x0 | zeros]
    xsT_d = nc.dram_tensor("xsT", (D, 2432), BF16, kind="ExternalInput").ap()
    Wq_d = nc.dram_tensor("Wq", (D, INNER), BF16, kind="ExternalInput").ap()
    Wk_d = nc.dram_tensor("Wk", (D, INNER), BF16, kind="ExternalInput").ap()
    Wv_d = nc.dram_tensor("Wv", (D, INNER), BF16, kind="ExternalInput").ap()
    Wo_d = nc.dram_tensor("Wo", (INNER, D), BF16, kind="ExternalInput").ap()
    bob_d = nc.dram_tensor("bob", (128, D), F32, kind="ExternalInput").ap()
    y_d = nc.dram_tensor("y", (QUART, D), F32, kind="ExternalOutput").ap()
    kTo_d = nc.dram_tensor("kTo", (2, 128, 4, NHALF), BF16, kind="ExternalOutput").ap()
    vo_d = nc.dram_tensor("vo", (2, 128, 8, VW * 8), BF16, kind="ExternalOutput").ap()

    # token-column ranges in xsT for (half, chunk): chunk 0 = [x0pad | 3 main
    # tiles], chunk 1 = 4 main tiles, chunk 2 = 3 main tiles. Half 1 reuses
    # half 0's projected k/v for its tiles 0-2 (global + the 2-tile halo
    # overlap), so its chunk 0 only fetches the columns q/k still need.
    def chunk_cols(hf, c):
        base = 1024 * hf
        if c == 0:
            if hf == 1:
                return 512, ((256, base + 128, 256),)
            return 512, ((0, 2304, 128), (128, base, 384))
        if c == 1:
            return 512, ((0, base + 384, 512),)
        return 384, ((0, base + 896, 384),)

    CHUNKS = [(hf, c) for hf in range(2) for c in range(3)]

    with tile.TileContext(nc) as tc:
        with (
            tc.tile_pool(name="xst", bufs=48) as xpool,
            tc.tile_pool(name="const", bufs=1) as constp,
            tc.tile_pool(name="qkv", bufs=1) as qkvp,
        ):
            xtiles = {}

            def prefetch(hf, c, eng=None):
                # descriptor generation is serial per issuing queue; TRN2 has
                # two HWDGE queues (SP + Activation), so chunks are spread
                # across both to halve the startup descgen latency
                eng = eng or nc.sync
                W, segs = chunk_cols(hf, c)
                tl = [
                    xpool.tile([128, W], BF16, tag="xt", name=f"xt{hf}_{c}_{d8}")
                    for d8 in range(8)
                ]
                # seg-major: the first compute group reads seg 0 (x0pad) of
                # every d8 tile, so those descriptors all come first
                for (o, src, w) in segs:
                    for d8 in range(8):
                        eng.dma_start(
                            tl[d8][:, o : o + w],
                            xsT_d[128 * d8 : 128 * d8 + 128, src : src + w],
                        )
                xtiles[(hf, c)] = tl

            # x tiles all stay resident (48 bufs): issue every chunk's DMA
            # upfront, interleaved with the weights, so no descriptor-gen or
            # ring-buffer wait ever blocks the stream mid-kernel. The first
            # matmul group needs wv + chunk-0 x, so those go first.
            wq = constp.tile([128, 8, INNER], BF16, name="wq")
            wk = constp.tile([128, 8, INNER], BF16, name="wk")
            wv = constp.tile([128, 8, INNER], BF16, name="wv")
            wo = constp.tile([128, 4, D], BF16, name="wo")
            # wv + chunk-0 x0pad gate the very first matmul group: interleave
            # them 1:1 across both descriptor-gen queues
            wvre = Wv_d.rearrange("(po pi) f -> pi po f", pi=128)
            for kt in range(8):
                eng = nc.scalar if kt % 2 else nc.sync
                eng.dma_start(wv[:, kt], wvre[:, kt])
            for w_r, w_d in ((wq, Wq_d), (wk, Wk_d)):
                wre = w_d.rearrange("(po pi) f -> pi po f", pi=128)
                for kt in range(8):
                    nc.scalar.dma_start(w_r[:, kt], wre[:, kt])
            prefetch(0, 0)
            prefetch(0, 1, nc.scalar)
            prefetch(0, 2)
            prefetch(1, 1, nc.scalar)
            prefetch(1, 0)
            prefetch(1, 2, nc.scalar)
            wore = Wo_d.rearrange("(po pi) f -> pi po f", pi=128)
            nc.scalar.dma_start(wo[:], wore)

            bias = constp.tile([128, D], F32)
            nc.sync.dma_start(bias[:], bob_d)

            ident = constp.tile([128, 128], BF16)
            make_identity(nc, ident[:])

            ones_col = constp.tile([128, 1], F32)
            nc.gpsimd.memset(ones_col[:], 1.0)
            zero_col = constp.tile([128, 1], F32)
            nc.gpsimd.memset(zero_col[:], 0.0)

            kv_prev = None
            for hf in range(2):
                if True:
                    qT = qkvp.tile([128, 4, SLAB], BF16, name="qT", tag="qT",
                                   bufs=1)
                    kT = qkvp.tile([128, 4, SLAB], BF16, name="kT", tag="kT",
                                   bufs=2)
                    v = qkvp.tile([128, NT, VW * 8], BF16, name="v", tag="v",
                                  bufs=2)
                    attT = qkvp.tile([128, 4, NHALF], BF16, name="attT",
                                     tag="attT", bufs=1)

                    # ---- ones columns of v ----
                    vsplit = v[:].rearrange("p t (h c) -> p t h c", c=VW)
                    nc.vector.tensor_copy(
                        vsplit[:, 1:NT, :, 64:66],
                        ones_col[:, None, None, :].to_broadcast((128, NT - 1, 8, 2)),
                    )
                    if hf == 0:
                        # tile 0 holds [x0; zeros]: only row 0 contributes to l
                        nc.vector.tensor_copy(
                            vsplit[:, 0, :, 64:66],
                            zero_col[:, None, :].to_broadcast((128, 8, 2)),
                        )
                        nc.vector.tensor_copy(
                            vsplit[0:1, 0, :, 64:66],
                            ones_col[0:1, None, :].to_broadcast((1, 8, 2)),
                        )
                    else:
                        # reuse half 0's projections: tile 0 (global x0pad)
                        # plus the 2-tile halo overlap (half0 tiles 9,10 =
                        # half1 tiles 1,2)
                        kT0, v0 = kv_prev
                        vs0 = v0[:].rearrange("p t (h c) -> p t h c", c=VW)
                        nc.vector.tensor_copy(vsplit[:, 0], vs0[:, 0])
                        nc.vector.tensor_copy(vsplit[:, 1:3], vs0[:, 9:11])
                        nc.vector.tensor_copy(kT[:, :, 0:128], kT0[:, :, 0:128])
                        nc.vector.tensor_copy(
                            kT[:, :, 128:384], kT0[:, :, 1152:1408]
                        )
                    kv_prev = (kT, v)

                    # ======== phase 1: projections (x^T streamed from host) ====
                    with (
                        tc.tile_pool(name=f"pp1{hf}", bufs=4, space="PSUM") as pps1,
                    ):
                        for c in range(3):
                            W = chunk_cols(hf, c)[0]
                            ntc = W // 128
                            s0 = 4 * c
                            xc = xtiles.pop((hf, c))
                            i0 = 3 if (hf == 1 and c == 0) else 0
                            # v first (phase 2 consumes it first)
                            for i in range(i0, ntc):
                                s = s0 + i
                                pp = pps1.tile([128, 512], F32, tag="pp")
                                for kt in range(8):
                                    nc.tensor.matmul(
                                        pp[:],
                                        xc[kt][:, 128 * i : 128 * i + 128],
                                        wv[:, kt, :],
                                        start=(kt == 0),
                                        stop=(kt == 7),
                                    )
                                nc.vector.tensor_copy(
                                    vsplit[:, s, :, 0:64],
                                    pp[:].rearrange("p (h c) -> p h c", c=64),
                                )
                            # k over the chunk (minus reused halo columns in
                            # half 1); q trimmed to real query blocks 2..9
                            if c == 0:
                                qo, qw = 256, 256
                                ko, kw = (384, 128) if hf == 1 else (0, W)
                            elif c == 1:
                                qo, qw = 0, 512
                                ko, kw = 0, W
                            else:
                                qo, qw = 0, 256
                                ko, kw = 0, W
                            for w_r, dstT, off, wd in (
                                (wk, kT, ko, kw),
                                (wq, qT, qo, qw),
                            ):
                                for mt in range(4):
                                    pp = pps1.tile([128, 512], F32, tag="pp")
                                    for kt in range(8):
                                        nc.tensor.matmul(
                                            pp[:, 0:wd],
                                            w_r[:, kt, 128 * mt : 128 * mt + 128],
                                            xc[kt][:, off : off + wd],
                                            start=(kt == 0),
                                            stop=(kt == 7),
                                        )
                                    nc.vector.tensor_copy(
                                        dstT[:, mt, 512 * c + off : 512 * c + off + wd],
                                        pp[:, 0:wd],
                                    )
                        # exports for the host-side global-token row (split
                        # so no single DMA queue is held for ~30us)
                        nc.sync.dma_start(kTo_d[hf][:, 0:2], kT[:, 0:2, 256:1280])
                        nc.sync.dma_start(kTo_d[hf][:, 2:4], kT[:, 2:4, 256:1280])
                        nc.sync.dma_start(vo_d[hf][:, 0:4], v[:, 2:6, :])
                        nc.sync.dma_start(vo_d[hf][:, 4:8], v[:, 6:10, :])

                    # ======== phase 2: block attention ========
                    # slabs of transposed scores s^T[k_tile, q_span]:
                    # index 0,1 = global tile 0 vs q-blocks 0-3 / 4-7
                    # index t+1 (t=1..10) = k-tile t vs 4 anchored q-blocks
                    def slab_info(idx):
                        # (k_col, q_col, in-slab col offset, width): only the
                        # columns of blocks that actually attend this k-tile
                        # are computed; the rest of the 512-wide slot is
                        # never read.
                        if idx < 2:
                            return 0, 128 * (4 * idx + 2), 0, 512
                        t = idx - 1
                        st = min(max(t - 4, 0), 4)
                        lo = max(t - 3, 0)
                        hi = min(t - 1, 7)
                        off = 128 * (lo - st)
                        return 128 * t, 128 * (st + 2), off, 128 * (hi - lo + 1)

                    def chunk_lhsT(pts, b, j):
                        # lhsT slice of p^T for block b, chunk j (-1 = global)
                        if j < 0:
                            idx = b // 4
                            coff = 128 * (b % 4)
                        else:
                            t = b + 1 + j
                            idx = t + 1
                            st = min(max(t - 4, 0), 4)
                            coff = 128 * (b - st)
                        hq, slot = divmod(idx, 2)
                        c0 = 512 * slot + coff
                        return pts[hq][:, c0 : c0 + 128]

                    # NOTE: PSUM accumulation groups must not interleave
                    # within one bank (start=True clobbers the bank), so
                    # each block gets its own single-bank og tile and its
                    # four chunks run back-to-back: j=0 (start), j=1, j=2,
                    # global (stop).
                    with (
                        tc.tile_pool(name=f"pt{hf}", bufs=7) as ptp,
                        tc.tile_pool(name=f"asb{hf}", bufs=16) as asbp,
                        tc.tile_pool(name=f"rr{hf}", bufs=3) as rrp,
                        tc.tile_pool(name=f"S{hf}", bufs=6, space="PSUM") as Sp,
                        tc.tile_pool(name=f"og{hf}", bufs=1, space="PSUM") as ogp,
                        tc.tile_pool(name=f"tp2{hf}", bufs=1, space="PSUM") as tp2p,
                    ):
                        att2 = {}
                        for h in range(H):
                            r0 = 64 * (h % 2)
                            mt_h = h // 2
                            hrows = slice(r0, r0 + 64)

                            ogs = {}
                            pts = []

                            def out_chunk(b, j, start, stop):
                                nc.tensor.matmul(
                                    ogs[b][:],
                                    chunk_lhsT(pts, b, j),
                                    v[:, 0 if j < 0 else b + 1 + j,
                                      VW * h : VW * h + VW],
                                    start=start,
                                    stop=stop,
                                )

                            def epilogue(b):
                                # adjacent heads share mt_h: stash even-head
                                # att, then transpose both heads' 64-col
                                # halves in one [128,128] PE transpose
                                og = ogs.pop(b)
                                r = rrp.tile([128, 1], F32, tag="rr")
                                nc.vector.reciprocal(r[:], og[:, 64:65])
                                if h % 2 == 0:
                                    a2 = asbp.tile([128, 128], BF16, tag="att",
                                                   name=f"a2_{hf}_{h}_{b}")
                                    att2[b] = a2
                                else:
                                    a2 = att2.pop(b)
                                nc.vector.tensor_tensor(
                                    a2[:, r0 : r0 + 64],
                                    og[:, 0:64],
                                    r[:].to_broadcast((128, 64)),
                                    MUL,
                                )
                                if h % 2 == 1:
                                    tp = tp2p.tile([128, 128], BF16, tag="tp2")
                                    nc.tensor.transpose(tp[:], a2[:], ident[:])
                                    nc.vector.tensor_copy(
                                        attT[:, mt_h, 128 * b : 128 * b + 128],
                                        tp[:],
                                    )

                            for hq in range(6):
                                # per-slot score tiles (one PSUM bank each) so
                                # exp of slot 0 overlaps the slot-1 matmul
                                pt = ptp.tile([128, 1024], BF16, tag="pt")
                                pts.append(pt)
                                for slot in range(2):
                                    kc, qc, off, wd = slab_info(2 * hq + slot)
                                    S = Sp.tile([128, 512], F32, tag="S")
                                    nc.tensor.matmul(
                                        S[:, off : off + wd],
                                        kT[hrows, mt_h, kc : kc + 128],
                                        qT[hrows, mt_h, qc + off : qc + off + wd],
                                        start=True,
                                        stop=True,
                                    )
                                    nc.scalar.activation(
                                        pt[:, 512 * slot + off : 512 * slot + off + wd],
                                        S[:, off : off + wd],
                                        EXPF,
                                        scale=SCALE,
                                    )

                                # blocks whose last k-slab (t = b+3) just
                                # became ready: run all four chunks
                                # back-to-back so only one og accumulation
                                # is ever in flight per head
                                for t in (2 * hq - 1, 2 * hq):
                                    b = t - 3
                                    if 0 <= b <= 7:
                                        ogs[b] = ogp.tile(
                                            [128, VW], F32, tag="og",
                                            name=f"og{hf}_{h}_{b}",
                                        )
                                        out_chunk(b, 0, True, False)
                                        out_chunk(b, 1, False, False)
                                        out_chunk(b, 2, False, False)
                                        out_chunk(b, -1, False, True)
                                        epilogue(b)

                    # ======== phase 3: output projection ========
                    with (
                        tc.tile_pool(name=f"ysb{hf}", bufs=3) as ysbp,
                        tc.tile_pool(name=f"yps{hf}", bufs=2, space="PSUM") as ypsp,
                    ):
                        for m in range(8):
                            yp = ypsp.tile([128, D], F32, tag="yp")
                            for kt in range(4):
                                lhsT = attT[:, kt, 128 * m : 128 * m + 128]
                                nc.tensor.matmul(
                                    yp[:, 0:512], lhsT, wo[:, kt, 0:512],
                                    start=(kt == 0), stop=(kt == 3),
                                )
                                nc.tensor.matmul(
                                    yp[:, 512:1024], lhsT, wo[:, kt, 512:1024],
                                    start=(kt == 0), stop=(kt == 3),
                                )
                            ysb = ysbp.tile([128, D], F32, tag="ysb")
                            nc.vector.tensor_tensor(ysb[:], yp[:], bias[:], ADD)
                            row = 1024 * hf + 128 * m
                            # split the very last tiles across queues so the
                            # final 512KB transfer doesn't serialize the tail,
                            # alternating the two descriptor-gen queues
                            nsp = 8 if (hf == 1 and m == 7) else (4 if (hf == 1 and m == 6) else 1)
                            cs = D // nsp
                            for sp in range(nsp):
                                eng = nc.scalar if (m + sp) % 2 else nc.sync
                                eng.dma_start(
                                    y_d[row : row + 128, sp * cs : (sp + 1) * cs],
                                    ysb[:, sp * cs : (sp + 1) * cs],
                                )

    nc.compile()
    return nc


def _get_nc():
    if "nc" not in _CACHE:
        _CACHE["nc"] = _build_nc()
    return _CACHE["nc"]


def kernel(x, Wq, Wk, Wv, Wo, bo):
    from concourse.bass_utils import run_bass_kernel_spmd
    from ml_dtypes import bfloat16

    x = np.ascontiguousarray(np.asarray(x, dtype=np.float32))
    Wq = np.ascontiguousarray(np.asarray(Wq, dtype=np.float32))
    Wk = np.ascontiguousarray(np.asarray(Wk, dtype=np.float32))
    Wv = np.ascontiguousarray(np.asarray(Wv, dtype=np.float32))
    Wo = np.ascontiguousarray(np.asarray(Wo, dtype=np.float32))
    bo = np.ascontiguousarray(np.asarray(bo, dtype=np.float32))

    # transposed zero-padded block-token sequence in bf16:
    # xpT[b, :, 128:8320] = x[b, 1:].T
    xb = x.astype(bfloat16)
    xpT = np.zeros((B, D, 8448), dtype=bfloat16)
    xpT[:, :, 128:8320] = xb.transpose(0, 2, 1)[:, :, 1:]
    bob = np.ascontiguousarray(np.broadcast_to(bo, (128, D)))
    Wqb = Wq.astype(bfloat16)
    Wkb = Wk.astype(bfloat16)
    Wvb = Wv.astype(bfloat16)
    Wob = Wo.astype(bfloat16)

    in_maps = []
    for c in range(8):
        bb, qi = divmod(c, 4)
        xsc = np.zeros((D, 2432), dtype=bfloat16)
        xsc[:, 0:2304] = xpT[bb, :, 2048 * qi : 2048 * qi + 2304]
        xsc[:, 2304] = xb[bb, 0]
        in_maps.append(
            {"xsT": xsc, "Wq": Wqb, "Wk": Wkb, "Wv": Wvb, "Wo": Wob, "bob": bob}
        )

    nc = _get_nc()
    trace = bool(int(os.environ.get("KERNEL_TRACE", "0")))
    res = run_bass_kernel_spmd(
        nc, in_maps, core_ids=list(range(8)), trace=trace
    )
    if trace and res.exec_time_ns is not None:
        _CACHE["exec_time_ns"] = res.exec_time_ns
        _CACHE["mean_exec_time_ns"] = res.mean_exec_time_ns
    outs = res.results

    y = np.empty((B, T, D), dtype=np.float32)
    for c in range(8):
        bb, qi = divmod(c, 4)
        y[bb, 1 + 2048 * qi : 1 + 2048 * (qi + 1)] = outs[c]["y"]

    # ---- global token row (host reduction over exported k/v) ----
    for bb in range(2):
        x0 = x[bb, 0].astype(np.float64)
        q0 = (x0 @ Wq.astype(np.float64)).reshape(H, DK)
        kg = (x0 @ Wk.astype(np.float64)).reshape(H, DK)
        vg = (x0 @ Wv.astype(np.float64)).reshape(H, DV)
        s00 = (q0 * kg).sum(1) * SCALE
        o = np.exp(s00)[:, None] * vg          # (H, DV)
        l = np.exp(s00)                        # (H,)
        for qi in range(4):
            out = outs[4 * bb + qi]
            for hfi in range(2):
                kTm = (
                    np.asarray(out["kTo"][hfi]).astype(np.float64)
                    .transpose(1, 0, 2).reshape(INNER, NHALF)
                )
                sg = (
                    np.einsum("hd,hdt->ht", q0, kTm.reshape(H, DK, NHALF))
                    * SCALE
                )
                p = np.exp(sg)                 # (H, NHALF)
                vt = np.asarray(out["vo"][hfi]).astype(np.float64)
                for h in range(H):
                    vh = (
                        vt[:, :, VW * h : VW * h + 64]
                        .transpose(1, 0, 2)
                        .reshape(NHALF, DV)
                    )
                    o[h] += p[h] @ vh
                    l[h] += p[h].sum()
        att0 = (o / l[:, None]).reshape(INNER)
        y[bb, 0] = (att0 @ Wo.astype(np.float64) + bo).astype(np.float32)

    return y
